# revision 9
# baseline (speedup 1.0000x reference)
"""Trainium2 Bass kernel for nn_CustomLoss_84043920048360.

Strategy (data-parallel over batch, 8 cores x 4 batches):
  The whole loss reduces to per-batch segment-sums over positions s:
    Q[j, c]      = sum_{s: target[s]==j} predicted[s, c]
    counts[j, c] = sum_{s: target[s]==j} [argmax_c' predicted[s, c'] == c]
    sumexp[s]    = sum_c exp(predicted[s, c])
  Q and counts come out of ONE TensorE bf16 matmul per 128-position chunk:
    lhsT = onehot(target) [s, j], rhs = [x_bf16 | onehot(argmax)] [s, 256]
  accumulated over 64 chunks in PSUM. The argmax one-hots are computed with
  full-f32 compares (bf16 only stores exact 0/1; x is cast to bf16 only for
  the Q matmul, ~0.2% noise on Q which feeds O(1)-scale means). sumexp
  ships to the host, which does lse=log(sumexp), the tiny [128]-sized
  mode/cipher/nll math in float64, and the final scalar combine. No
  collectives are needed.

Position mapping within a 1024-position block: s = it*1024 + p*8 + g
(p = SBUF partition, g = chunk-in-iter) so each partition's DMA is one
contiguous 4 KiB run.
"""

import os
import numpy as np

B, S, C = 32, 8192, 128
NCORES = 8
B_LOC = B // NCORES          # 4 batches per core
G = 8                        # chunks per iteration
CHUNK = 128                  # positions per chunk (matmul K)
ITERS = S // (G * CHUNK)     # 8 iterations per batch
NCHUNK = S // CHUNK          # 64 chunks per batch
N_ACT = 6                    # chunks/iter whose sumexp rides ACT accum (rest: DVE)

_cache = {}


def _build(b_loc=B_LOC, iters=ITERS, n_act=N_ACT):
    import concourse.bacc as bacc
    import concourse.tile as tile
    from concourse import mybir

    f32 = mybir.dt.float32
    bf16 = mybir.dt.bfloat16
    s_loc = iters * G * CHUNK

    nc = bacc.Bacc(
        "TRN2", target_bir_lowering=False, debug=False, num_devices=NCORES
    )
    pred = nc.dram_tensor("predicted", [b_loc, s_loc, C], f32, kind="ExternalInput")
    tgt = nc.dram_tensor("target_f32", [b_loc, s_loc], f32, kind="ExternalInput")
    iota = nc.dram_tensor("iota_f32", [128, 128], f32, kind="ExternalInput")
    q_out = nc.dram_tensor("q_out", [b_loc, 128, 256], f32, kind="ExternalOutput")
    se_out = nc.dram_tensor(
        "se_out", [b_loc, 128, iters * G], f32, kind="ExternalOutput"
    )

    # s = it*(G*128) + p*G + g
    pv = pred.ap().rearrange("b (i p g) c -> b i p g c", i=iters, p=128, g=G)
    tv = tgt.ap().rearrange("b (i p g) -> b i p g", i=iters, p=128, g=G)

    AX = mybir.AxisListType.X
    EQ = mybir.AluOpType.is_equal

    with tile.TileContext(nc) as tc:
        with (
            tc.tile_pool(name="consts", bufs=1) as consts,
            tc.tile_pool(name="work", bufs=3) as work,
            tc.tile_pool(name="acc", bufs=2) as accp,
            tc.tile_pool(name="psum", bufs=2, space="PSUM") as psum,
            tc.tile_pool(name="outp", bufs=2) as outp,
        ):
            iota_sb = consts.tile([128, 128], f32)
            nc.sync.dma_start(iota_sb[:], iota.ap())

            for b in range(b_loc):
                ps = psum.tile([128, 256], f32, tag="ps")
                se_sb = accp.tile([128, iters * G], f32, tag="se")
                for it in range(iters):
                    xt = work.tile([128, G, 128], f32, tag="xt")
                    rhs = work.tile([128, G, 256], bf16, tag="rhs")
                    tg = work.tile([128, G], f32, tag="tg")
                    oht = work.tile([128, G, 128], bf16, tag="oht")
                    e_t = work.tile([128, G, 128], f32, tag="e")
                    rmax = work.tile([128, G], f32, tag="rmax")

                    nc.sync.dma_start(xt[:], pv[b, it])
                    nc.sync.dma_start(tg[:], tv[b, it])

                    # cast x -> bf16 for the Q matmul columns (gpsimd)
                    nc.gpsimd.tensor_copy(rhs[:, :, 0:128], xt[:])
                    # rowmax over classes
                    nc.vector.reduce_max(rmax[:], xt[:], axis=AX)
                    for g in range(G):
                        # onehot(target): iota[c] == target  (exact int compare)
                        nc.vector.tensor_scalar(
                            out=oht[:, g, :],
                            in0=iota_sb[:],
                            scalar1=tg[:, g : g + 1],
                            scalar2=None,
                            op0=EQ,
                        )
                        # onehot(argmax): full-f32 compare, bf16 0/1 out
                        nc.vector.tensor_scalar(
                            out=rhs[:, g, 128:256],
                            in0=xt[:, g, :],
                            scalar1=rmax[:, g : g + 1],
                            scalar2=None,
                            op0=EQ,
                        )
                        # sumexp for the first n_act chunks rides ACT's accum
                        if g < n_act:
                            nc.scalar.activation(
                                e_t[:, g, :],
                                xt[:, g, :],
                                mybir.ActivationFunctionType.Exp,
                                accum_out=se_sb[:, it * G + g : it * G + g + 1],
                            )
                    if n_act < G:
                        nc.scalar.activation(
                            e_t[:, n_act:, :],
                            xt[:, n_act:, :],
                            mybir.ActivationFunctionType.Exp,
                        )
                        nc.vector.reduce_sum(
                            se_sb[:, it * G + n_act : (it + 1) * G],
                            e_t[:, n_act:, :],
                            axis=AX,
                        )
                    # segment-sum matmul: psum += onehot(target).T @ [x|onehot(am)]
                    for g in range(G):
                        nc.tensor.matmul(
                            ps[:],
                            oht[:, g, :],
                            rhs[:, g, :],
                            start=(it == 0 and g == 0),
                            stop=(it == iters - 1 and g == G - 1),
                        )
                q_sb = outp.tile([128, 256], f32, tag="q")
                nc.scalar.copy(q_sb[:], ps[:])
                nc.sync.dma_start(q_out.ap()[b], q_sb[:])
                nc.sync.dma_start(se_out.ap()[b], se_sb[:])

    nc.compile()
    return nc


def _get_nc(b_loc=B_LOC, iters=ITERS):
    key = (b_loc, iters)
    if key not in _cache:
        _cache[key] = _build(b_loc, iters)
    return _cache[key]


_IOTA = np.tile(np.arange(128, dtype=np.float32), (128, 1))
last_results = None


def _run_device(predicted, target):
    """predicted [B,S,C] f32, target [B,S] int -> (q [B,128,256], se [B,S]) float64"""
    from concourse.bass_utils import run_bass_kernel_spmd

    nc = _get_nc()
    tgt_f32 = target.astype(np.float32)
    in_maps = []
    for core in range(NCORES):
        b0 = core * B_LOC
        in_maps.append(
            {
                "predicted": np.ascontiguousarray(predicted[b0 : b0 + B_LOC]),
                "target_f32": np.ascontiguousarray(tgt_f32[b0 : b0 + B_LOC]),
                "iota_f32": _IOTA,
            }
        )
    global last_results
    last_results = run_bass_kernel_spmd(
        nc, in_maps, core_ids=list(range(NCORES))
    )
    q = np.concatenate([r["q_out"] for r in last_results.results], axis=0)
    se = np.concatenate([r["se_out"] for r in last_results.results], axis=0)
    # se[b, p, it*G+g] -> sumexp[b, s] with s = it*(G*128) + p*G + g
    se = (
        se.reshape(B, 128, ITERS, G)
        .transpose(0, 2, 1, 3)
        .reshape(B, S)
    )
    return q.astype(np.float64), se.astype(np.float64)


def kernel(predicted, target):
    predicted = np.asarray(predicted)
    target = np.asarray(target)
    in_dtype = predicted.dtype
    q, se = _run_device(predicted.astype(np.float32, copy=False), target)

    total_cipher = 0.0
    total_nz = 0
    total_gather = 0.0
    for b in range(B):
        Q = q[b, :, 0:128]          # [j, c] segment sums of x (bf16 inputs)
        counts = q[b, :, 128:256]   # [j, c] argmax histogram (exact)
        t_b = target[b].astype(np.int64)
        lse = np.log(se[b])
        n_eq = np.bincount(t_b, minlength=C).astype(np.float64)
        Lt = np.bincount(t_b, weights=lse, minlength=C)
        L = lse.sum()
        mode = np.argmax(counts, axis=1)
        P = Q.sum(axis=0)
        Qg = Q[np.arange(C), mode]
        Pg = P[mode]
        sum_all = L - Pg
        sum_eq = Lt - Qg
        sum_ne = sum_all - sum_eq
        ne_cnt = S - n_eq
        eq_mean = sum_eq / np.maximum(n_eq, 1.0)
        ne_mean = sum_ne / np.maximum(ne_cnt, 1.0)
        inv_ne = np.where(ne_cnt > 0, 1.0 / np.maximum(ne_mean, 1e-30), 0.0)
        cipher = np.where(n_eq > 0, 0.5 * eq_mean + 0.5 * inv_ne, 0.0)
        total_cipher += cipher.sum()
        total_nz += int((cipher != 0).sum())
        total_gather += Q[np.arange(C), np.arange(C)].sum()

    cipher_mean = total_cipher / max(total_nz, 1)
    nll = -total_gather / (B * S)
    out = 0.5 * cipher_mean + 0.5 * nll
    out_dtype = in_dtype if in_dtype in (np.float32, np.float64) else np.float32
    return np.asarray(out, dtype=out_dtype)


# revision 12
# speedup vs baseline: 1.1859x; 1.1859x over previous
"""Trainium2 Bass kernel for nn_CustomLoss_84043920048360.

Strategy (data-parallel over batch, 8 cores x 4 batches):
  The whole loss reduces to per-batch segment-sums over positions s:
    Q[j, c]      = sum_{s: target[s]==j} predicted[s, c]
    counts[j, c] = sum_{s: target[s]==j} [argmax_c' predicted[s, c'] == c]
    sumexp[s]    = sum_c exp(predicted[s, c])
  Q and counts come out of ONE TensorE bf16 matmul per 128-position chunk:
    lhsT = onehot(target) [s, j], rhs = [x_bf16 | onehot(argmax)] [s, 256]
  accumulated over 64 chunks in PSUM. The argmax one-hots are computed with
  full-f32 compares (bf16 only stores exact 0/1; x is cast to bf16 only for
  the Q matmul, ~0.2% noise on Q which feeds O(1)-scale means). sumexp
  ships to the host, which does lse=log(sumexp), the tiny [128]-sized
  mode/cipher/nll math in float64, and the final scalar combine. No
  collectives are needed.

Position mapping within a 1024-position block: s = it*1024 + p*8 + g
(p = SBUF partition, g = chunk-in-iter) so each partition's DMA is one
contiguous 4 KiB run.
"""

import os
import numpy as np

B, S, C = 32, 8192, 128
NCORES = 8
B_LOC = B // NCORES          # 4 batches per core
G = 8                        # chunks per iteration
CHUNK = 128                  # positions per chunk (matmul K)
ITERS = S // (G * CHUNK)     # 8 iterations per batch
NCHUNK = S // CHUNK          # 64 chunks per batch
N_ACT = 6                    # chunks/iter whose sumexp rides ACT accum (rest: DVE)

_cache = {}


def _build(b_loc=B_LOC, iters=ITERS, n_act=N_ACT):
    import concourse.bacc as bacc
    import concourse.tile as tile
    from concourse import mybir

    f32 = mybir.dt.float32
    bf16 = mybir.dt.bfloat16
    s_loc = iters * G * CHUNK

    nc = bacc.Bacc(
        "TRN2", target_bir_lowering=False, debug=False, num_devices=NCORES
    )
    pred = nc.dram_tensor("predicted", [b_loc, s_loc, C], f32, kind="ExternalInput")
    tgt = nc.dram_tensor("target_f32", [b_loc, s_loc], f32, kind="ExternalInput")
    iota = nc.dram_tensor("iota_bf16", [128, 128], bf16, kind="ExternalInput")
    ident = nc.dram_tensor("ident_bf16", [128, 128], bf16, kind="ExternalInput")
    q_out = nc.dram_tensor("q_out", [b_loc, 128, 256], f32, kind="ExternalOutput")
    se_out = nc.dram_tensor(
        "se_out", [b_loc, 128, iters * G], f32, kind="ExternalOutput"
    )

    # s = it*(G*128) + p*G + g
    pv = pred.ap().rearrange("b (i p g) c -> b i p g c", i=iters, p=128, g=G)
    tv = tgt.ap().rearrange("b (i p g) -> b i p g", i=iters, p=128, g=G)

    AX = mybir.AxisListType.X
    EQ = mybir.AluOpType.is_equal

    with tile.TileContext(nc) as tc:
        with (
            tc.tile_pool(name="consts", bufs=1) as consts,
            tc.tile_pool(name="work", bufs=3) as work,
            tc.tile_pool(name="psum", bufs=2, space="PSUM") as psum,
        ):
            iota_sb = consts.tile([128, 128], bf16)
            nc.sync.dma_start(iota_sb[:], iota.ap())
            ident_sb = consts.tile([128, 128], bf16)
            nc.sync.dma_start(ident_sb[:], ident.ap())
            ones_sb = consts.tile([128, 1], bf16)
            nc.vector.memset(ones_sb[:], 1.0)

            for b in range(b_loc):
                ps = psum.tile([128, 256], f32, tag="ps")
                se_ps = psum.tile([128, iters * G], f32, tag="se")
                for it in range(iters):
                    xt = work.tile([128, G, 128], f32, tag="xt")
                    rhs = work.tile([128, G, 256], bf16, tag="rhs")
                    tg = work.tile([128, G], f32, tag="tg")
                    oht = work.tile([128, G, 128], bf16, tag="oht")
                    eT = work.tile([128, G, 128], bf16, tag="e")
                    rmax = work.tile([128, G], f32, tag="rmax")
                    xT = psum.tile([128, G, 128], bf16, tag="xT")

                    nc.sync.dma_start(xt[:], pv[b, it])
                    nc.sync.dma_start(tg[:], tv[b, it])

                    # cast x -> bf16 for the Q matmul columns (gpsimd)
                    nc.gpsimd.tensor_copy(rhs[:, :, 0:128], xt[:])
                    # rowmax over classes
                    nc.vector.reduce_max(rmax[:], xt[:], axis=AX)
                    for g in range(G):
                        # onehot(target): iota[c] == target (bf16 int compare, 4x)
                        nc.vector.tensor_scalar(
                            out=oht[:, g, :],
                            in0=iota_sb[:],
                            scalar1=tg[:, g : g + 1],
                            scalar2=None,
                            op0=EQ,
                        )
                        # onehot(argmax): full-f32 compare, bf16 0/1 out
                        nc.vector.tensor_scalar(
                            out=rhs[:, g, 128:256],
                            in0=xt[:, g, :],
                            scalar1=rmax[:, g : g + 1],
                            scalar2=None,
                            op0=EQ,
                        )
                        # transpose x_bf16 chunk into PSUM: xT[c, s]
                        nc.tensor.transpose(
                            xT[:, g, :], rhs[:, g, 0:128], ident_sb[:]
                        )
                    # exp on the transposed tile (PSUM -> SBUF, one op)
                    nc.scalar.activation(
                        eT[:], xT[:], mybir.ActivationFunctionType.Exp
                    )
                    for g in range(G):
                        # sumexp[s] = ones @ eT = column sums (PE, N=1)
                        nc.tensor.matmul(
                            se_ps[:, it * G + g : it * G + g + 1],
                            eT[:, g, :],
                            ones_sb[:],
                            start=True,
                            stop=True,
                        )
                        # segment-sum matmul: psum += oht.T @ [x|onehot(am)]
                        nc.tensor.matmul(
                            ps[:],
                            oht[:, g, :],
                            rhs[:, g, :],
                            start=(it == 0 and g == 0),
                            stop=(it == iters - 1 and g == G - 1),
                        )
                q_sb = work.tile([128, 256], f32, tag="q")
                nc.scalar.copy(q_sb[:], ps[:])
                nc.sync.dma_start(q_out.ap()[b], q_sb[:])
                se_sb = work.tile([128, iters * G], f32, tag="sesb")
                nc.scalar.copy(se_sb[:], se_ps[:])
                nc.sync.dma_start(se_out.ap()[b], se_sb[:])

    nc.compile()
    return nc


def _get_nc(b_loc=B_LOC, iters=ITERS):
    key = (b_loc, iters)
    if key not in _cache:
        _cache[key] = _build(b_loc, iters)
    return _cache[key]


import ml_dtypes

_IOTA = np.tile(np.arange(128), (128, 1)).astype(ml_dtypes.bfloat16)
_IDENT = np.eye(128).astype(ml_dtypes.bfloat16)
last_results = None


def _run_device(predicted, target):
    """predicted [B,S,C] f32, target [B,S] int -> (q [B,128,256], se [B,S]) float64"""
    from concourse.bass_utils import run_bass_kernel_spmd

    nc = _get_nc()
    tgt_f32 = target.astype(np.float32)
    in_maps = []
    for core in range(NCORES):
        b0 = core * B_LOC
        in_maps.append(
            {
                "predicted": np.ascontiguousarray(predicted[b0 : b0 + B_LOC]),
                "target_f32": np.ascontiguousarray(tgt_f32[b0 : b0 + B_LOC]),
                "iota_bf16": _IOTA,
                "ident_bf16": _IDENT,
            }
        )
    global last_results
    last_results = run_bass_kernel_spmd(
        nc, in_maps, core_ids=list(range(NCORES))
    )
    q = np.concatenate([r["q_out"] for r in last_results.results], axis=0)
    se = np.concatenate([r["se_out"] for r in last_results.results], axis=0)
    # se[b, p, it*G+g] -> sumexp[b, s] with s = it*(G*128) + p*G + g
    se = (
        se.reshape(B, 128, ITERS, G)
        .transpose(0, 2, 1, 3)
        .reshape(B, S)
    )
    return q.astype(np.float64), se.astype(np.float64)


def kernel(predicted, target):
    predicted = np.asarray(predicted)
    target = np.asarray(target)
    in_dtype = predicted.dtype
    q, se = _run_device(predicted.astype(np.float32, copy=False), target)

    total_cipher = 0.0
    total_nz = 0
    total_gather = 0.0
    for b in range(B):
        Q = q[b, :, 0:128]          # [j, c] segment sums of x (bf16 inputs)
        counts = q[b, :, 128:256]   # [j, c] argmax histogram (exact)
        t_b = target[b].astype(np.int64)
        lse = np.log(se[b])
        n_eq = np.bincount(t_b, minlength=C).astype(np.float64)
        Lt = np.bincount(t_b, weights=lse, minlength=C)
        L = lse.sum()
        mode = np.argmax(counts, axis=1)
        P = Q.sum(axis=0)
        Qg = Q[np.arange(C), mode]
        Pg = P[mode]
        sum_all = L - Pg
        sum_eq = Lt - Qg
        sum_ne = sum_all - sum_eq
        ne_cnt = S - n_eq
        eq_mean = sum_eq / np.maximum(n_eq, 1.0)
        ne_mean = sum_ne / np.maximum(ne_cnt, 1.0)
        inv_ne = np.where(ne_cnt > 0, 1.0 / np.maximum(ne_mean, 1e-30), 0.0)
        cipher = np.where(n_eq > 0, 0.5 * eq_mean + 0.5 * inv_ne, 0.0)
        total_cipher += cipher.sum()
        total_nz += int((cipher != 0).sum())
        total_gather += Q[np.arange(C), np.arange(C)].sum()

    cipher_mean = total_cipher / max(total_nz, 1)
    nll = -total_gather / (B * S)
    out = 0.5 * cipher_mean + 0.5 * nll
    out_dtype = in_dtype if in_dtype in (np.float32, np.float64) else np.float32
    return np.asarray(out, dtype=out_dtype)


# revision 17
# speedup vs baseline: 1.4837x; 1.2512x over previous
"""Trainium2 Bass kernel for nn_CustomLoss_84043920048360.

Strategy (data-parallel over batch, 8 cores x 4 batches):
  The whole loss reduces to per-batch segment-sums over positions s:
    Q[j, c]      = sum_{s: target[s]==j} predicted[s, c]
    counts[j, c] = sum_{s: target[s]==j} [argmax_c' predicted[s, c'] == c]
    sumexp[s]    = sum_c exp(predicted[s, c])
  Q and counts come out of ONE TensorE bf16 matmul per 128-position chunk:
    lhsT = onehot(target) [s, j], rhs = [x_bf16 | onehot(argmax)] [s, 256]
  accumulated over 64 chunks in PSUM. The argmax one-hots are computed with
  full-f32 compares (bf16 only stores exact 0/1; x is cast to bf16 only for
  the Q matmul, ~0.2% noise on Q which feeds O(1)-scale means). sumexp
  ships to the host, which does lse=log(sumexp), the tiny [128]-sized
  mode/cipher/nll math in float64, and the final scalar combine. No
  collectives are needed.

Position mapping within a 1024-position block: s = it*1024 + p*8 + g
(p = SBUF partition, g = chunk-in-iter) so each partition's DMA is one
contiguous 4 KiB run.
"""

import os
import numpy as np

B, S, C = 32, 8192, 128
NCORES = 8
B_LOC = B // NCORES          # 4 batches per core
G = 8                        # chunks per iteration
CHUNK = 128                  # positions per chunk (matmul K)
ITERS = S // (G * CHUNK)     # 8 iterations per batch
NCHUNK = S // CHUNK          # 64 chunks per batch
N_ACT = 6                    # chunks/iter whose sumexp rides ACT accum (rest: DVE)

_cache = {}


def _build(b_loc=B_LOC, iters=ITERS, n_pool=1, drop_se=False, drop_tr=False):
    import concourse.bacc as bacc
    import concourse.tile as tile
    from concourse import mybir

    f32 = mybir.dt.float32
    bf16 = mybir.dt.bfloat16
    s_loc = iters * G * CHUNK

    nc = bacc.Bacc(
        "TRN2", target_bir_lowering=False, debug=False, num_devices=NCORES
    )
    fp8 = mybir.dt.float8e4
    pred = nc.dram_tensor("predicted", [b_loc, s_loc, C], f32, kind="ExternalInput")
    oht_in = nc.dram_tensor("oht_fp8", [b_loc, s_loc, C], fp8, kind="ExternalInput")
    ident = nc.dram_tensor("ident_fp8", [128, 128], fp8, kind="ExternalInput")
    q_out = nc.dram_tensor("q_out", [b_loc, 128, 256], f32, kind="ExternalOutput")
    se_out = nc.dram_tensor(
        "se_out", [b_loc, 128, iters * G], f32, kind="ExternalOutput"
    )

    # s = it*(G*128) + p*G + g
    pv = pred.ap().rearrange("b (i p g) c -> b i p g c", i=iters, p=128, g=G)
    ov8 = oht_in.ap().rearrange("b (i p g) c -> b i p g c", i=iters, p=128, g=G)

    AX = mybir.AxisListType.X
    EQ = mybir.AluOpType.is_equal

    with tile.TileContext(nc) as tc:
        with (
            tc.tile_pool(name="consts", bufs=1) as consts,
            tc.tile_pool(name="work", bufs=3) as work,
            tc.tile_pool(name="psum", bufs=2, space="PSUM") as psum,
        ):
            ident_sb = consts.tile([128, 128], fp8)
            nc.sync.dma_start(ident_sb[:], ident.ap())
            ones_sb = consts.tile([128, 1], bf16)
            nc.vector.memset(ones_sb[:], 1.0)

            for b in range(b_loc):
                ps = psum.tile([128, 256], f32, tag="ps")
                se_ps = psum.tile([128, iters * G], f32, tag="se")
                for it in range(iters):
                    xt = work.tile([128, G, 128], f32, tag="xt")
                    rhs = work.tile([128, G, 256], fp8, tag="rhs")
                    oht = work.tile([128, G, 128], fp8, tag="oht")
                    eT = work.tile([128, G, 128], bf16, tag="e")
                    rmax = work.tile([128, G], f32, tag="rmax")
                    xT = psum.tile([128, G, 256], fp8, tag="xT")

                    nc.sync.dma_start(xt[:], pv[b, it])
                    nc.sync.dma_start(oht[:], ov8[b, it])

                    # cast x -> fp8 for the Q matmul + transpose path (gpsimd)
                    nc.gpsimd.tensor_copy(rhs[:, :, 0:128], xt[:])
                    # rowmax over classes
                    nc.vector.reduce_max(rmax[:], xt[:], axis=AX)
                    for g in range(G):
                        # onehot(argmax): full-f32 compare, fp8 0/1 out
                        eng = nc.gpsimd if g < n_pool else nc.vector
                        eng.tensor_scalar(
                            out=rhs[:, g, 128:256],
                            in0=xt[:, g, :],
                            scalar1=rmax[:, g : g + 1],
                            scalar2=None,
                            op0=EQ,
                        )
                        # transpose x_fp8 chunk into PSUM: xT[c, s]
                        if not drop_tr:
                            nc.tensor.transpose(
                                xT[:, g, 0:256:2], rhs[:, g, 0:128], ident_sb[:]
                            )
                    # exp on the transposed tile (PSUM -> SBUF, one op)
                    if not drop_tr:
                        nc.scalar.activation(
                            eT[:], xT[:, :, 0:256:2], mybir.ActivationFunctionType.Exp
                        )
                    for g in range(G):
                        # sumexp[s] = ones @ eT = column sums (PE, N=1)
                        if not drop_se:
                            nc.tensor.matmul(
                                se_ps[:, it * G + g : it * G + g + 1],
                                eT[:, g, :],
                                ones_sb[:],
                                start=True,
                                stop=True,
                            )
                        # segment-sum matmul: psum += oht.T @ [x|onehot(am)]
                        nc.tensor.matmul(
                            ps[:],
                            oht[:, g, :],
                            rhs[:, g, :],
                            start=(it == 0 and g == 0),
                            stop=(it == iters - 1 and g == G - 1),
                        )
                q_sb = work.tile([128, 256], f32, tag="q")
                nc.scalar.copy(q_sb[:], ps[:])
                nc.sync.dma_start(q_out.ap()[b], q_sb[:])
                se_sb = work.tile([128, iters * G], f32, tag="sesb")
                nc.scalar.copy(se_sb[:], se_ps[:])
                nc.sync.dma_start(se_out.ap()[b], se_sb[:])

    nc.compile()
    return nc


def _get_nc(b_loc=B_LOC, iters=ITERS):
    key = (b_loc, iters)
    if key not in _cache:
        _cache[key] = _build(b_loc, iters)
    return _cache[key]


import ml_dtypes

_FP8 = ml_dtypes.float8_e4m3
_IDENT8 = np.eye(128).astype(_FP8)
_EYE8 = np.eye(128).astype(_FP8)
last_results = None


def _run_device(predicted, target):
    """predicted [B,S,C] f32, target [B,S] int -> (q [B,128,256], se [B,S]) float64"""
    from concourse.bass_utils import run_bass_kernel_spmd

    nc = _get_nc()
    oht8 = _EYE8[target.astype(np.int64)]
    in_maps = []
    for core in range(NCORES):
        b0 = core * B_LOC
        in_maps.append(
            {
                "predicted": np.ascontiguousarray(predicted[b0 : b0 + B_LOC]),
                "oht_fp8": np.ascontiguousarray(oht8[b0 : b0 + B_LOC]),
                "ident_fp8": _IDENT8,
            }
        )
    global last_results
    last_results = run_bass_kernel_spmd(
        nc, in_maps, core_ids=list(range(NCORES))
    )
    q = np.concatenate([r["q_out"] for r in last_results.results], axis=0)
    se = np.concatenate([r["se_out"] for r in last_results.results], axis=0)
    # se[b, p, it*G+g] -> sumexp[b, s] with s = it*(G*128) + p*G + g
    se = (
        se.reshape(B, 128, ITERS, G)
        .transpose(0, 2, 1, 3)
        .reshape(B, S)
    )
    return q.astype(np.float64), se.astype(np.float64)


def kernel(predicted, target):
    predicted = np.asarray(predicted)
    target = np.asarray(target)
    in_dtype = predicted.dtype
    q, se = _run_device(predicted.astype(np.float32, copy=False), target)

    total_cipher = 0.0
    total_nz = 0
    total_gather = 0.0
    for b in range(B):
        Q = q[b, :, 0:128]          # [j, c] segment sums of x (bf16 inputs)
        counts = q[b, :, 128:256]   # [j, c] argmax histogram (exact)
        t_b = target[b].astype(np.int64)
        lse = np.log(se[b])
        n_eq = np.bincount(t_b, minlength=C).astype(np.float64)
        Lt = np.bincount(t_b, weights=lse, minlength=C)
        L = lse.sum()
        mode = np.argmax(counts, axis=1)
        P = Q.sum(axis=0)
        Qg = Q[np.arange(C), mode]
        Pg = P[mode]
        sum_all = L - Pg
        sum_eq = Lt - Qg
        sum_ne = sum_all - sum_eq
        ne_cnt = S - n_eq
        eq_mean = sum_eq / np.maximum(n_eq, 1.0)
        ne_mean = sum_ne / np.maximum(ne_cnt, 1.0)
        inv_ne = np.where(ne_cnt > 0, 1.0 / np.maximum(ne_mean, 1e-30), 0.0)
        cipher = np.where(n_eq > 0, 0.5 * eq_mean + 0.5 * inv_ne, 0.0)
        total_cipher += cipher.sum()
        total_nz += int((cipher != 0).sum())
        total_gather += Q[np.arange(C), np.arange(C)].sum()

    cipher_mean = total_cipher / max(total_nz, 1)
    nll = -total_gather / (B * S)
    out = 0.5 * cipher_mean + 0.5 * nll
    out_dtype = in_dtype if in_dtype in (np.float32, np.float64) else np.float32
    return np.asarray(out, dtype=out_dtype)


# revision 22
# speedup vs baseline: 1.6118x; 1.0864x over previous
"""Trainium2 Bass kernel for nn_CustomLoss_84043920048360.

Strategy (data-parallel over batch, 8 cores x 4 batches):
  The whole loss reduces to per-batch segment-sums over positions s:
    Q[j, c]      = sum_{s: target[s]==j} predicted[s, c]
    counts[j, c] = sum_{s: target[s]==j} [argmax_c' predicted[s, c'] == c]
    sumexp[s]    = sum_c exp(predicted[s, c])
  Q and counts come out of ONE TensorE bf16 matmul per 128-position chunk:
    lhsT = onehot(target) [s, j], rhs = [x_bf16 | onehot(argmax)] [s, 256]
  accumulated over 64 chunks in PSUM. The argmax one-hots are computed with
  full-f32 compares (bf16 only stores exact 0/1; x is cast to bf16 only for
  the Q matmul, ~0.2% noise on Q which feeds O(1)-scale means). sumexp
  ships to the host, which does lse=log(sumexp), the tiny [128]-sized
  mode/cipher/nll math in float64, and the final scalar combine. No
  collectives are needed.

Position mapping within a 1024-position block: s = it*1024 + p*8 + g
(p = SBUF partition, g = chunk-in-iter) so each partition's DMA is one
contiguous 4 KiB run.
"""

import os
import numpy as np

B, S, C = 32, 8192, 128
NCORES = 8
B_LOC = B // NCORES          # 4 batches per core
G = 8                        # chunks per iteration
CHUNK = 128                  # positions per chunk (matmul K)
ITERS = S // (G * CHUNK)     # 8 iterations per batch
NCHUNK = S // CHUNK          # 64 chunks per batch
N_ACT = 6                    # chunks/iter whose sumexp rides ACT accum (rest: DVE)

_cache = {}


def _build(b_loc=B_LOC, iters=ITERS, n_pool=2, drop_se=False, drop_tr=False, wbufs=3, pbufs=2, sep_x=True):
    import concourse.bacc as bacc
    import concourse.tile as tile
    from concourse import mybir

    f32 = mybir.dt.float32
    bf16 = mybir.dt.bfloat16
    s_loc = iters * G * CHUNK

    nc = bacc.Bacc(
        "TRN2", target_bir_lowering=False, debug=False, num_devices=NCORES
    )
    pred = nc.dram_tensor("predicted", [b_loc, s_loc, C], bf16, kind="ExternalInput")
    oht_in = nc.dram_tensor("oht_bf16", [b_loc, s_loc, C], bf16, kind="ExternalInput")
    ident = nc.dram_tensor("ident_bf16", [128, 128], bf16, kind="ExternalInput")
    q_out = nc.dram_tensor("q_out", [b_loc, 128, 256], f32, kind="ExternalOutput")
    se_out = nc.dram_tensor(
        "se_out", [b_loc, 128, iters * G], f32, kind="ExternalOutput"
    )

    # s = it*(G*128) + p*G + g
    pv = pred.ap().rearrange("b (i p g) c -> b i p g c", i=iters, p=128, g=G)
    ov8 = oht_in.ap().rearrange("b (i p g) c -> b i p g c", i=iters, p=128, g=G)

    AX = mybir.AxisListType.X
    EQ = mybir.AluOpType.is_equal

    with tile.TileContext(nc) as tc:
        with (
            tc.tile_pool(name="consts", bufs=1) as consts,
            tc.tile_pool(name="work", bufs=wbufs) as work,
            tc.tile_pool(name="psum", bufs=pbufs, space="PSUM") as psum,
        ):
            ident_sb = consts.tile([128, 128], bf16)
            nc.sync.dma_start(ident_sb[:], ident.ap())
            ones_sb = consts.tile([128, 1], bf16)
            nc.vector.memset(ones_sb[:], 1.0)

            for b in range(b_loc):
                ps = psum.tile([128, 256], f32, tag="ps")
                se_ps = psum.tile([128, iters * G], f32, tag="se")
                for it in range(iters):
                    rhs = work.tile([128, G, 256], bf16, tag="rhs")
                    oht = work.tile([128, G, 128], bf16, tag="oht")
                    eT = work.tile([128, G, 128], bf16, tag="e")
                    rmax = work.tile([128, G], f32, tag="rmax")
                    xT = psum.tile([128, G, 128], bf16, tag="xT")

                    if sep_x:
                        xb_t = work.tile([128, G, 128], bf16, tag="xbt")
                        nc.sync.dma_start(xb_t[:], pv[b, it])
                        u32 = mybir.dt.uint32
                        nc.gpsimd.tensor_copy(
                            rhs[:, :, 0:128].bitcast(u32), xb_t[:].bitcast(u32)
                        )
                        x_src = xb_t
                    else:
                        nc.sync.dma_start(rhs[:, :, 0:128], pv[b, it])
                        x_src = None
                    nc.sync.dma_start(oht[:], ov8[b, it])

                    # rowmax over classes (bf16 in, f32 out)
                    nc.vector.reduce_max(
                        rmax[:],
                        (x_src[:] if sep_x else rhs[:, :, 0:128]),
                        axis=AX,
                    )
                    for g in range(G):
                        # onehot(argmax): full-f32 compare, fp8 0/1 out
                        eng = nc.gpsimd if g < n_pool else nc.vector
                        eng.tensor_scalar(
                            out=rhs[:, g, 128:256],
                            in0=(x_src[:, g, :] if sep_x else rhs[:, g, 0:128]),
                            scalar1=rmax[:, g : g + 1],
                            scalar2=None,
                            op0=EQ,
                        )
                        # transpose x_fp8 chunk into PSUM: xT[c, s]
                        if not drop_tr:
                            nc.tensor.transpose(
                                xT[:, g, :], rhs[:, g, 0:128], ident_sb[:]
                            )
                    # exp on the transposed tile (PSUM -> SBUF, one op)
                    if not drop_tr:
                        nc.scalar.activation(
                            eT[:], xT[:], mybir.ActivationFunctionType.Exp
                        )
                    for g in range(G):
                        # sumexp[s] = ones @ eT = column sums (PE, N=1)
                        if not drop_se:
                            nc.tensor.matmul(
                                se_ps[:, it * G + g : it * G + g + 1],
                                eT[:, g, :],
                                ones_sb[:],
                                start=True,
                                stop=True,
                            )
                        # segment-sum matmul: psum += oht.T @ [x|onehot(am)]
                        nc.tensor.matmul(
                            ps[:],
                            oht[:, g, :],
                            rhs[:, g, :],
                            start=(it == 0 and g == 0),
                            stop=(it == iters - 1 and g == G - 1),
                        )
                q_sb = work.tile([128, 256], f32, tag="q")
                nc.scalar.copy(q_sb[:], ps[:])
                nc.sync.dma_start(q_out.ap()[b], q_sb[:])
                se_sb = work.tile([128, iters * G], f32, tag="sesb")
                nc.scalar.copy(se_sb[:], se_ps[:])
                nc.sync.dma_start(se_out.ap()[b], se_sb[:])

    nc.compile()
    return nc


def _get_nc(b_loc=B_LOC, iters=ITERS):
    key = (b_loc, iters)
    if key not in _cache:
        _cache[key] = _build(b_loc, iters)
    return _cache[key]


import ml_dtypes

_FP8 = ml_dtypes.float8_e4m3
_BF16 = ml_dtypes.bfloat16
_IDENT = np.eye(128).astype(_BF16)
_EYE = np.eye(128).astype(_BF16)
last_results = None


def _run_device(predicted, target):
    """predicted [B,S,C] f32, target [B,S] int -> (q [B,128,256], se [B,S]) float64"""
    from concourse.bass_utils import run_bass_kernel_spmd

    nc = _get_nc()
    xb = predicted.astype(_BF16)
    ohtb = _EYE[target.astype(np.int64)]
    in_maps = []
    for core in range(NCORES):
        b0 = core * B_LOC
        in_maps.append(
            {
                "predicted": np.ascontiguousarray(xb[b0 : b0 + B_LOC]),
                "oht_bf16": np.ascontiguousarray(ohtb[b0 : b0 + B_LOC]),
                "ident_bf16": _IDENT,
            }
        )
    global last_results
    last_results = run_bass_kernel_spmd(
        nc, in_maps, core_ids=list(range(NCORES))
    )
    q = np.concatenate([r["q_out"] for r in last_results.results], axis=0)
    se = np.concatenate([r["se_out"] for r in last_results.results], axis=0)
    # se[b, p, it*G+g] -> sumexp[b, s] with s = it*(G*128) + p*G + g
    se = (
        se.reshape(B, 128, ITERS, G)
        .transpose(0, 2, 1, 3)
        .reshape(B, S)
    )
    return q.astype(np.float64), se.astype(np.float64)


def kernel(predicted, target):
    predicted = np.asarray(predicted)
    target = np.asarray(target)
    in_dtype = predicted.dtype
    q, se = _run_device(predicted.astype(np.float32, copy=False), target)

    total_cipher = 0.0
    total_nz = 0
    total_gather = 0.0
    for b in range(B):
        Q = q[b, :, 0:128]          # [j, c] segment sums of x (bf16 inputs)
        counts = q[b, :, 128:256]   # [j, c] argmax histogram (exact)
        t_b = target[b].astype(np.int64)
        lse = np.log(se[b])
        n_eq = np.bincount(t_b, minlength=C).astype(np.float64)
        Lt = np.bincount(t_b, weights=lse, minlength=C)
        L = lse.sum()
        mode = np.argmax(counts, axis=1)
        P = Q.sum(axis=0)
        Qg = Q[np.arange(C), mode]
        Pg = P[mode]
        sum_all = L - Pg
        sum_eq = Lt - Qg
        sum_ne = sum_all - sum_eq
        ne_cnt = S - n_eq
        eq_mean = sum_eq / np.maximum(n_eq, 1.0)
        ne_mean = sum_ne / np.maximum(ne_cnt, 1.0)
        inv_ne = np.where(ne_cnt > 0, 1.0 / np.maximum(ne_mean, 1e-30), 0.0)
        cipher = np.where(n_eq > 0, 0.5 * eq_mean + 0.5 * inv_ne, 0.0)
        total_cipher += cipher.sum()
        total_nz += int((cipher != 0).sum())
        total_gather += Q[np.arange(C), np.arange(C)].sum()

    cipher_mean = total_cipher / max(total_nz, 1)
    nll = -total_gather / (B * S)
    out = 0.5 * cipher_mean + 0.5 * nll
    out_dtype = in_dtype if in_dtype in (np.float32, np.float64) else np.float32
    return np.asarray(out, dtype=out_dtype)


# revision 23
# speedup vs baseline: 1.7039x; 1.0571x over previous
"""Trainium2 Bass kernel for nn_CustomLoss_84043920048360.

Strategy (data-parallel over batch, 8 cores x 4 batches):
  The whole loss reduces to per-batch segment-sums over positions s:
    Q[j, c]      = sum_{s: target[s]==j} predicted[s, c]
    counts[j, c] = sum_{s: target[s]==j} [argmax_c' predicted[s, c'] == c]
    sumexp[s]    = sum_c exp(predicted[s, c])
  Q and counts come out of ONE TensorE bf16 matmul per 128-position chunk:
    lhsT = onehot(target) [s, j], rhs = [x_bf16 | onehot(argmax)] [s, 256]
  accumulated over 64 chunks in PSUM. The argmax one-hots are computed with
  full-f32 compares (bf16 only stores exact 0/1; x is cast to bf16 only for
  the Q matmul, ~0.2% noise on Q which feeds O(1)-scale means). sumexp
  ships to the host, which does lse=log(sumexp), the tiny [128]-sized
  mode/cipher/nll math in float64, and the final scalar combine. No
  collectives are needed.

Position mapping within a 1024-position block: s = it*1024 + p*8 + g
(p = SBUF partition, g = chunk-in-iter) so each partition's DMA is one
contiguous 4 KiB run.
"""

import os
import numpy as np

B, S, C = 32, 8192, 128
NCORES = 8
B_LOC = B // NCORES          # 4 batches per core
G = 8                        # chunks per iteration
CHUNK = 128                  # positions per chunk (matmul K)
ITERS = S // (G * CHUNK)     # 8 iterations per batch
NCHUNK = S // CHUNK          # 64 chunks per batch
N_ACT = 6                    # chunks/iter whose sumexp rides ACT accum (rest: DVE)

_cache = {}


def _build(b_loc=B_LOC, iters=ITERS, n_pool=2, drop_se=False, drop_tr=False, wbufs=4, pbufs=2, sep_x=True):
    import concourse.bacc as bacc
    import concourse.tile as tile
    from concourse import mybir

    f32 = mybir.dt.float32
    bf16 = mybir.dt.bfloat16
    s_loc = iters * G * CHUNK

    nc = bacc.Bacc(
        "TRN2", target_bir_lowering=False, debug=False, num_devices=NCORES
    )
    pred = nc.dram_tensor("predicted", [b_loc, s_loc, C], bf16, kind="ExternalInput")
    oht_in = nc.dram_tensor("oht_bf16", [b_loc, s_loc, C], bf16, kind="ExternalInput")
    ident = nc.dram_tensor("ident_bf16", [128, 128], bf16, kind="ExternalInput")
    q_out = nc.dram_tensor("q_out", [b_loc, 128, 256], f32, kind="ExternalOutput")
    se_out = nc.dram_tensor(
        "se_out", [b_loc, 128, iters * G], f32, kind="ExternalOutput"
    )

    # s = it*(G*128) + p*G + g
    pv = pred.ap().rearrange("b (i p g) c -> b i p g c", i=iters, p=128, g=G)
    ov8 = oht_in.ap().rearrange("b (i p g) c -> b i p g c", i=iters, p=128, g=G)

    AX = mybir.AxisListType.X
    EQ = mybir.AluOpType.is_equal

    with tile.TileContext(nc) as tc:
        with (
            tc.tile_pool(name="consts", bufs=1) as consts,
            tc.tile_pool(name="work", bufs=wbufs) as work,
            tc.tile_pool(name="psum", bufs=pbufs, space="PSUM") as psum,
        ):
            ident_sb = consts.tile([128, 128], bf16)
            nc.sync.dma_start(ident_sb[:], ident.ap())
            ones_sb = consts.tile([128, 1], bf16)
            nc.vector.memset(ones_sb[:], 1.0)

            for b in range(b_loc):
                ps = psum.tile([128, 256], f32, tag="ps")
                se_ps = psum.tile([128, iters * G], f32, tag="se")
                for it in range(iters):
                    rhs = work.tile([128, G, 256], bf16, tag="rhs")
                    oht = work.tile([128, G, 128], bf16, tag="oht")
                    eT = work.tile([128, G, 128], bf16, tag="e")
                    rmax = work.tile([128, G], f32, tag="rmax")
                    xT = psum.tile([128, G, 128], bf16, tag="xT")

                    if sep_x:
                        xb_t = work.tile([128, G, 128], bf16, tag="xbt")
                        nc.sync.dma_start(xb_t[:], pv[b, it])
                        u32 = mybir.dt.uint32
                        nc.gpsimd.tensor_copy(
                            rhs[:, :, 0:128].bitcast(u32), xb_t[:].bitcast(u32)
                        )
                        x_src = xb_t
                    else:
                        nc.sync.dma_start(rhs[:, :, 0:128], pv[b, it])
                        x_src = None
                    nc.sync.dma_start(oht[:], ov8[b, it])

                    # rowmax over classes (bf16 in, f32 out)
                    nc.vector.reduce_max(
                        rmax[:],
                        (x_src[:] if sep_x else rhs[:, :, 0:128]),
                        axis=AX,
                    )
                    for g in range(G):
                        # onehot(argmax): full-f32 compare, fp8 0/1 out
                        eng = nc.gpsimd if g < n_pool else nc.vector
                        eng.tensor_scalar(
                            out=rhs[:, g, 128:256],
                            in0=(x_src[:, g, :] if sep_x else rhs[:, g, 0:128]),
                            scalar1=rmax[:, g : g + 1],
                            scalar2=None,
                            op0=EQ,
                        )
                        # transpose x_fp8 chunk into PSUM: xT[c, s]
                        if not drop_tr:
                            nc.tensor.transpose(
                                xT[:, g, :], rhs[:, g, 0:128], ident_sb[:]
                            )
                    # exp on the transposed tile (PSUM -> SBUF, one op)
                    if not drop_tr:
                        nc.scalar.activation(
                            eT[:], xT[:], mybir.ActivationFunctionType.Exp
                        )
                    for g in range(G):
                        # sumexp[s] = ones @ eT = column sums (PE, N=1)
                        if not drop_se:
                            nc.tensor.matmul(
                                se_ps[:, it * G + g : it * G + g + 1],
                                eT[:, g, :],
                                ones_sb[:],
                                start=True,
                                stop=True,
                            )
                        # segment-sum matmul: psum += oht.T @ [x|onehot(am)]
                        nc.tensor.matmul(
                            ps[:],
                            oht[:, g, :],
                            rhs[:, g, :],
                            start=(it == 0 and g == 0),
                            stop=(it == iters - 1 and g == G - 1),
                        )
                q_sb = work.tile([128, 256], f32, tag="q")
                nc.scalar.copy(q_sb[:], ps[:])
                nc.sync.dma_start(q_out.ap()[b], q_sb[:])
                se_sb = work.tile([128, iters * G], f32, tag="sesb")
                nc.scalar.copy(se_sb[:], se_ps[:])
                nc.sync.dma_start(se_out.ap()[b], se_sb[:])

    nc.compile()
    return nc


def _get_nc(b_loc=B_LOC, iters=ITERS):
    key = (b_loc, iters)
    if key not in _cache:
        _cache[key] = _build(b_loc, iters)
    return _cache[key]


import ml_dtypes

_FP8 = ml_dtypes.float8_e4m3
_BF16 = ml_dtypes.bfloat16
_IDENT = np.eye(128).astype(_BF16)
_EYE = np.eye(128).astype(_BF16)
last_results = None


def _run_device(predicted, target):
    """predicted [B,S,C] f32, target [B,S] int -> (q [B,128,256], se [B,S]) float64"""
    from concourse.bass_utils import run_bass_kernel_spmd

    nc = _get_nc()
    xb = predicted.astype(_BF16)
    ohtb = _EYE[target.astype(np.int64)]
    in_maps = []
    for core in range(NCORES):
        b0 = core * B_LOC
        in_maps.append(
            {
                "predicted": np.ascontiguousarray(xb[b0 : b0 + B_LOC]),
                "oht_bf16": np.ascontiguousarray(ohtb[b0 : b0 + B_LOC]),
                "ident_bf16": _IDENT,
            }
        )
    global last_results
    last_results = run_bass_kernel_spmd(
        nc, in_maps, core_ids=list(range(NCORES))
    )
    q = np.concatenate([r["q_out"] for r in last_results.results], axis=0)
    se = np.concatenate([r["se_out"] for r in last_results.results], axis=0)
    # se[b, p, it*G+g] -> sumexp[b, s] with s = it*(G*128) + p*G + g
    se = (
        se.reshape(B, 128, ITERS, G)
        .transpose(0, 2, 1, 3)
        .reshape(B, S)
    )
    return q.astype(np.float64), se.astype(np.float64)


def kernel(predicted, target):
    predicted = np.asarray(predicted)
    target = np.asarray(target)
    in_dtype = predicted.dtype
    q, se = _run_device(predicted.astype(np.float32, copy=False), target)

    total_cipher = 0.0
    total_nz = 0
    total_gather = 0.0
    for b in range(B):
        Q = q[b, :, 0:128]          # [j, c] segment sums of x (bf16 inputs)
        counts = q[b, :, 128:256]   # [j, c] argmax histogram (exact)
        t_b = target[b].astype(np.int64)
        lse = np.log(se[b])
        n_eq = np.bincount(t_b, minlength=C).astype(np.float64)
        Lt = np.bincount(t_b, weights=lse, minlength=C)
        L = lse.sum()
        mode = np.argmax(counts, axis=1)
        P = Q.sum(axis=0)
        Qg = Q[np.arange(C), mode]
        Pg = P[mode]
        sum_all = L - Pg
        sum_eq = Lt - Qg
        sum_ne = sum_all - sum_eq
        ne_cnt = S - n_eq
        eq_mean = sum_eq / np.maximum(n_eq, 1.0)
        ne_mean = sum_ne / np.maximum(ne_cnt, 1.0)
        inv_ne = np.where(ne_cnt > 0, 1.0 / np.maximum(ne_mean, 1e-30), 0.0)
        cipher = np.where(n_eq > 0, 0.5 * eq_mean + 0.5 * inv_ne, 0.0)
        total_cipher += cipher.sum()
        total_nz += int((cipher != 0).sum())
        total_gather += Q[np.arange(C), np.arange(C)].sum()

    cipher_mean = total_cipher / max(total_nz, 1)
    nll = -total_gather / (B * S)
    out = 0.5 * cipher_mean + 0.5 * nll
    out_dtype = in_dtype if in_dtype in (np.float32, np.float64) else np.float32
    return np.asarray(out, dtype=out_dtype)
